# revision 5
# baseline (speedup 1.0000x reference)
"""Trainium2 Bass kernel for nn_CustomLoss_188978561648.

loss = -(1/K) * sum_{k,i} num[k,i] / (var + rs[k,i] - num[k,i])
  rs  = zs @ X.T          [K, N]   (the dominant GEMM)
  num = zs * diag(X)      [K, N]

Sharding: tensor-parallel over the output columns i (rows of X).
Core c owns i in [c*512, (c+1)*512).

v3 design (from ~24.9us v2; measured 23328-24363 ns across 5
validation runs, spread driven by the HAM clock-phase lottery, see
below):
- fp8e4 (e4m3) matmul operands + MatmulPerfMode.DoubleRow; X diag-
  zeroed on host so the GEMM computes rs - num directly; +var folded
  in as a rank-1 fp16 matmul that opens the accumulation group.
- xt streamed in 1,2,2,4,4,2,1-pair blocks instead of big-first
  blocks: the first DR matmul starts ~2us earlier (gated by only
  zst+128KB instead of zst+512KB) and the matmul chain then tracks
  the stream at fine granularity early (where gating binds; the PE
  falls ~60ns/pair behind at half clock, so 4-pair middle blocks
  cost nothing and save 2 DMA issues + 2 serial ~160ns block-end
  waits); the 1-pair last block trims the final gate (+bytes ->
  +900ns sem -> last mm) by ~400ns. ~2-pair is
  the issue-rate sweet spot: one DMA_DIRECT2D issue costs ~610ns of
  sync-engine time, so all-1-pair blocks cap the stream at ~210GB/s
  (descriptor starvation, measured +2-4us on V2), while 4-pair
  blocks re-coarsen the gating (measured: first mm waits to 13.1us).
  Do NOT lead the ring with sub-KB-row DMAs (72/128B descriptors
  delayed first stream bytes by ~1.3us, measured).
- num shipped fp8e4m3 (was fp16): rel err rises 1.5e-5 -> 6.0e-4
  (gate 2e-2), saves 32KB and halves the scalar-ring traffic that
  competes for the SHARED 16 DMA engines (both HWDGE rings and SWDGE
  feed the same E64-79 engines; per-engine ~20-25GB/s => ~330GB/s
  per-core plateau, matching hw_specs DMA_UTILIZATION 0.83 * 400).
- epilogue: rcp_approx_fast + one accumulating STT -> red[64,1],
  cross-partition PE reduce vs a (-1/K) ones vector -> [1,1], DVE
  copy to SBUF; the final 4-byte DMA is issued AFTER the tile
  block-end (sync HWDGE + manual then_inc; sync single-packet issue
  ~670ns vs scalar ~1090ns, and the issuer is the last engine to
  join the final barrier) so the body never waits
  the ~900ns SEM_PROP_DMA for its completion -- the walrus teardown
  (~7us of full-sem-file resets, unavoidable compiler postamble)
  gives the write ample time to land before the NEFF retires.

Measured-on-HW notes (do not regress these):
- exec_time_ns = (end of last teardown instruction) - (first const
  MEMSET of the framework preamble). It INCLUDES ~1.2us framework
  preamble and ~7-8.3us walrus teardown (257 per-semaphore resets
  split across all 5 engines at ~115ns each + barriers). The
  teardown is emitted by walrus codegen (InstGroupResetSemaphores),
  is independent of queue declarations (shrinking nc.m.queues
  num_queues changed nothing), and cannot be shortened from BIR.
- DO NOT output multi-partition results directly: a [64,1] fp32
  store's 16 completion-sem increments trickle in over ~5.6us
  (measured), stalling the block end. Keep the PE reduce + [1,1]
  single_packet store.
- HWDGE sem completion: bytes -> wait release costs ~900ns
  (SEM_PROP_DMA_OVERHEAD_NS); every DMA-gated dependency pays it.
- The HAM governor runs the core at half clock (1.2GHz "others",
  matmul 630ns) in 3413ns quanta, granting 1-2 full-clock quanta
  (matmul 379ns) at a phase the kernel cannot control; both PE and
  the DMA engines ride it. This is the dominant run-to-run variance
  (grant observed anywhere in 15.4-21.8us). Warm-up matmuls make it
  strictly worse (+1.3us, measured in the previous session).
- exec ~= 1.2us preamble + 1.5us DGE startup + ~8us stream +
  ~1.5us matmul tail + ~1.8us epilogue + ~1.2us block-end/issue +
  ~7-8us teardown.
"""

import numpy as np

K = 64          # schedules (zs rows)
N = 4096        # channel dim
NCORES = 8
SHARD = N // NCORES            # 512 output columns per core
NCHUNKS = N // 128             # 32 contraction chunks of 128
NPAIRS = NCHUNKS // 2          # 16 DoubleRow chunk pairs
PAIR_BLOCKS = (1, 2, 2, 4, 4, 2, 1)
XCOLS = NCHUNKS * SHARD        # 16384 packed xt cols per partition

_CACHE = {}


def _build():
    import concourse.bacc as bacc
    import concourse.tile as tile
    import concourse.mybir as mybir
    f32 = mybir.dt.float32
    f16 = mybir.dt.float16
    f8 = mybir.dt.float8e4

    nc = bacc.Bacc(
        "TRN2", target_bir_lowering=False, debug=False, num_devices=NCORES
    )

    varrow_d = nc.dram_tensor("varrow", [1, K + SHARD], f16, kind="ExternalInput")
    zst_d = nc.dram_tensor("zst", [128, NCHUNKS * K], f8, kind="ExternalInput")
    xt_d = nc.dram_tensor("xt", [128, XCOLS], f8, kind="ExternalInput")
    num_d = nc.dram_tensor("num", [K, SHARD], f8, kind="ExternalInput")
    out_d = nc.dram_tensor("out", [1, 1], f32, kind="ExternalOutput")

    out_sb = nc.alloc_sbuf_tensor("out_sb", [1, 1], f32)

    with tile.TileContext(nc) as tc:
        with (
            tc.tile_pool(name="data", bufs=1) as dpool,
            tc.tile_pool(name="ep", bufs=1) as epool,
            tc.tile_pool(name="ps", bufs=1, space="PSUM") as pspool,
        ):
            # NOTE: no PE warm-up dummies (HAM duty governor, see module
            # docstring).
            ones_t = epool.tile([K, 1], f32, tag="ones")
            nc.vector.memset(ones_t[:], -1.0 / K)

            # -- stream: sync ring carries zst then the xt blocks (in
            #    matmul-consumption order); the tiny varrow + num ride
            #    the scalar ring so the sync ring's descriptor-issue
            #    slots are all spent on the 2.3MB stream. --
            varrow_t = dpool.tile([1, K + SHARD], f16, tag="varrow")
            nc.scalar.dma_start(varrow_t[:], varrow_d[:])
            zst_t = dpool.tile([128, NCHUNKS, K], f8, tag="zst")
            nc.sync.dma_start(zst_t[:], zst_d[:])
            xt_t = []
            off = 0
            for b, npair in enumerate(PAIR_BLOCKS):
                cols = npair * 2 * SHARD
                t = dpool.tile([128, npair * 2, SHARD], f8, tag=f"xt{b}")
                nc.sync.dma_start(t[:], xt_d[:, off : off + cols])
                xt_t.append(t)
                off += cols
            num_t = epool.tile([K, SHARD], f8, tag="num")
            nc.scalar.dma_start(num_t[:], num_d[:])

            # -- PE: +var rank-1 matmul opens the accumulation group,
            #    then 16 fp8 DoubleRow pair matmuls, one per 2-pair
            #    block half --
            ps = pspool.tile([K, SHARD], f32, tag="ps")
            nc.tensor.matmul(
                ps[:],
                varrow_t[:, :K],
                varrow_t[:, K:],
                start=True,
                stop=False,
                skip_group_check=True,
            )
            j = 0
            for b, npair in enumerate(PAIR_BLOCKS):
                for jj in range(npair):
                    nc.tensor.matmul(
                        ps[:],
                        zst_t[:, 2 * j : 2 * j + 2, :],
                        xt_t[b][:, 2 * jj : 2 * jj + 2, :],
                        start=False,
                        stop=(j == NPAIRS - 1),
                        perf_mode=mybir.MatmulPerfMode.DoubleRow,
                        skip_group_check=True,
                    )
                    j += 1

            # -- epilogue: PSUM holds den = var + rs - num --
            rcp_t = epool.tile([K, SHARD], f32, tag="rcp")
            scr_t = epool.tile([K, SHARD], f16, tag="scr")
            red_t = epool.tile([K, 1], f32, tag="red")
            nc.vector.reciprocal_approx_fast(rcp_t[:], ps[:])
            nc.vector.scalar_tensor_tensor(
                out=scr_t[:], in0=num_t[:], scalar=1.0, in1=rcp_t[:],
                op0=mybir.AluOpType.mult, op1=mybir.AluOpType.mult,
                accum_out=red_t[:],
            )
            # cross-partition reduce on PE: out = red.T @ (-1/K * ones)
            ps1 = pspool.tile([1, 1], f32, tag="ps1")
            nc.tensor.matmul(ps1[:], red_t[:], ones_t[:], start=True, stop=True)
            nc.vector.tensor_copy(out_sb.ap(), ps1[:])

    # Post-tile out DMA (see module docstring): issued after the tile
    # block-end barrier (which guarantees out_sb is written); the body
    # does not wait for its completion semaphore.
    out_sem = nc.alloc_semaphore("out_sem")
    nc.sync.dma_start(out_d[:], out_sb.ap(), single_packet=True).then_inc(
        out_sem, 16
    )

    nc.compile()
    return nc


def _prep_inputs(zs, X, var_noise):
    """Host-side shard + layout packing (layout + dtype cast; the only
    math is diag extraction, the zs*diag elementwise product, and
    zeroing X's diagonal)."""
    import ml_dtypes

    f8 = ml_dtypes.float8_e4m3
    zs = np.ascontiguousarray(np.asarray(zs, dtype=np.float32))
    X = np.asarray(X, dtype=np.float32)
    var = np.float32(np.asarray(var_noise).reshape(()))

    diag = np.ascontiguousarray(np.diagonal(X)).astype(np.float32)
    Xz = X.copy()
    np.fill_diagonal(Xz, 0.0)

    # zst[p, m, k] = zs[k, 128m + p], replicated across cores
    zst = np.ascontiguousarray(
        zs.reshape(K, NCHUNKS, 128).transpose(2, 1, 0)
    ).astype(f8).reshape(128, NCHUNKS * K)

    varrow = np.empty((1, K + SHARD), dtype=np.float16)
    varrow[0, :K] = var
    varrow[0, K:] = 1.0

    in_maps = []
    for c in range(NCORES):
        sl = slice(c * SHARD, (c + 1) * SHARD)
        # xt[p, m, il] = Xz[c*512 + il, 128m + p]
        xt = np.ascontiguousarray(
            Xz[sl].reshape(SHARD, NCHUNKS, 128).transpose(2, 1, 0)
        ).astype(f8).reshape(128, XCOLS)
        num = (zs[:, sl] * diag[sl][None, :]).astype(f8)
        in_maps.append({"varrow": varrow, "zst": zst, "xt": xt, "num": num})
    return in_maps


def _run(in_maps, **run_kwargs):
    from concourse.bass_utils import run_bass_kernel_spmd

    if "nc" not in _CACHE:
        _CACHE["nc"] = _build()
    nc = _CACHE["nc"]
    return run_bass_kernel_spmd(
        nc, in_maps, core_ids=list(range(NCORES)), **run_kwargs
    )


def kernel(zs, X, var_noise):
    in_maps = _prep_inputs(zs, X, var_noise)
    res = None
    for attempt in range(3):
        try:
            res = _run(in_maps).results
            break
        except Exception:
            if attempt == 2:
                raise
            import time

            time.sleep(2)
    total = np.float32(0.0)
    for c in range(NCORES):
        total += res[c]["out"].astype(np.float32).sum(dtype=np.float32)
    return np.float32(total)


# revision 6
# speedup vs baseline: 1.0621x; 1.0621x over previous
"""Trainium2 Bass kernel for nn_CustomLoss_188978561648.

loss = -(1/K) * sum_{k,i} num[k,i] / (var + rs[k,i] - num[k,i])
  rs  = zs @ X.T          [K, N]   (the dominant GEMM)
  num = zs * diag(X)      [K, N]

Sharding: tensor-parallel over the output columns i (rows of X).
Core c owns i in [c*512, (c+1)*512).

v3 design (from ~24.9us v2; measured 23328 / 23791 / 24037 / 25072
ns across validation runs of this structure, spread driven by the
HAM clock-phase lottery, see below):
- fp8e4 (e4m3) matmul operands + MatmulPerfMode.DoubleRow; X diag-
  zeroed on host so the GEMM computes rs - num directly; +var folded
  in as a rank-1 fp16 matmul that opens the accumulation group.
- xt streamed in 1,2,2,4,4,2,1-pair blocks instead of big-first
  blocks: the first DR matmul starts ~2us earlier (gated by only
  zst+128KB instead of zst+512KB) and the matmul chain then tracks
  the stream at fine granularity early (where gating binds; the PE
  falls ~60ns/pair behind at half clock, so 4-pair middle blocks
  cost nothing and save 2 DMA issues + 2 serial ~160ns block-end
  waits); the 1-pair last block trims the final gate (+bytes ->
  +900ns sem -> last mm) by ~400ns. ~2-pair is
  the issue-rate sweet spot: one DMA_DIRECT2D issue costs ~610ns of
  sync-engine time, so all-1-pair blocks cap the stream at ~210GB/s
  (descriptor starvation, measured +2-4us on V2), while 4-pair
  blocks re-coarsen the gating (measured: first mm waits to 13.1us).
  Do NOT lead the ring with sub-KB-row DMAs (72/128B descriptors
  delayed first stream bytes by ~1.3us, measured).
- num shipped fp8e4m3 (was fp16): rel err rises 1.5e-5 -> 6.0e-4
  (gate 2e-2), saves 32KB and halves the scalar-ring traffic that
  competes for the SHARED 16 DMA engines (both HWDGE rings and SWDGE
  feed the same E64-79 engines; per-engine ~20-25GB/s => ~330GB/s
  per-core plateau, matching hw_specs DMA_UTILIZATION 0.83 * 400).
- epilogue: rcp_approx_fast + one accumulating STT -> red[64,1],
  cross-partition PE reduce vs a (-1/K) ones vector -> [1,1], DVE
  copy to SBUF; the final 4-byte DMA is issued AFTER the tile
  block-end (sync HWDGE + manual then_inc; sync single-packet issue
  ~670ns vs scalar ~1090ns, and the issuer is the last engine to
  join the final barrier) so the body never waits
  the ~900ns SEM_PROP_DMA for its completion -- the walrus teardown
  (~7us of full-sem-file resets, unavoidable compiler postamble)
  gives the write ample time to land before the NEFF retires.

Measured-on-HW notes (do not regress these):
- exec_time_ns = (end of last teardown instruction) - (first const
  MEMSET of the framework preamble). It INCLUDES ~1.2us framework
  preamble and ~7-8.3us walrus teardown (257 per-semaphore resets
  split across all 5 engines at ~115ns each + barriers). The
  teardown is emitted by walrus codegen (InstGroupResetSemaphores),
  is independent of queue declarations (shrinking nc.m.queues
  num_queues changed nothing), and cannot be shortened from BIR.
- DO NOT output multi-partition results directly: a [64,1] fp32
  store's 16 completion-sem increments trickle in over ~5.6us
  (measured), stalling the block end. Keep the PE reduce + [1,1]
  single_packet store.
- HWDGE sem completion: bytes -> wait release costs ~900ns
  (SEM_PROP_DMA_OVERHEAD_NS); every DMA-gated dependency pays it.
- The HAM governor runs the core at half clock (1.2GHz "others",
  matmul 630ns) in 3413ns quanta, granting 1-2 full-clock quanta
  (matmul 379ns) at a phase the kernel cannot control; both PE and
  the DMA engines ride it. This is the dominant run-to-run variance
  (grant observed anywhere in 15.4-21.8us). Warm-up matmuls make it
  strictly worse (+1.3us, measured in the previous session).
- exec ~= 1.2us preamble + 1.5us DGE startup + ~8us stream +
  ~1.5us matmul tail + ~1.8us epilogue + ~1.2us block-end/issue +
  ~7-8us teardown.
"""

import numpy as np

K = 64          # schedules (zs rows)
N = 4096        # channel dim
NCORES = 8
SHARD = N // NCORES            # 512 output columns per core
NCHUNKS = N // 128             # 32 contraction chunks of 128
NPAIRS = NCHUNKS // 2          # 16 DoubleRow chunk pairs
PAIR_BLOCKS = (1, 2, 2, 4, 4, 2, 1)
XCOLS = NCHUNKS * SHARD        # 16384 packed xt cols per partition

_CACHE = {}


def _build():
    import concourse.bacc as bacc
    import concourse.tile as tile
    import concourse.mybir as mybir
    f32 = mybir.dt.float32
    f16 = mybir.dt.float16
    f8 = mybir.dt.float8e4

    nc = bacc.Bacc(
        "TRN2", target_bir_lowering=False, debug=False, num_devices=NCORES
    )

    varrow_d = nc.dram_tensor("varrow", [1, K + SHARD], f16, kind="ExternalInput")
    zst_d = nc.dram_tensor("zst", [128, NCHUNKS * K], f8, kind="ExternalInput")
    xt_d = nc.dram_tensor("xt", [128, XCOLS], f8, kind="ExternalInput")
    num_d = nc.dram_tensor("num", [K, SHARD], f8, kind="ExternalInput")
    out_d = nc.dram_tensor("out", [1, 1], f32, kind="ExternalOutput")

    out_sb = nc.alloc_sbuf_tensor("out_sb", [1, 1], f32)

    with tile.TileContext(nc) as tc:
        with (
            tc.tile_pool(name="data", bufs=1) as dpool,
            tc.tile_pool(name="ep", bufs=1) as epool,
            tc.tile_pool(name="ps", bufs=1, space="PSUM") as pspool,
        ):
            # NOTE: no PE warm-up dummies (HAM duty governor, see module
            # docstring).
            ones_t = epool.tile([K, 1], f32, tag="ones")
            nc.vector.memset(ones_t[:], -1.0 / K)

            # -- stream: sync ring carries zst then the xt blocks (in
            #    matmul-consumption order); the tiny varrow + num ride
            #    the scalar ring so the sync ring's descriptor-issue
            #    slots are all spent on the 2.3MB stream. --
            varrow_t = dpool.tile([1, K + SHARD], f16, tag="varrow")
            nc.scalar.dma_start(varrow_t[:], varrow_d[:])
            zst_t = dpool.tile([128, NCHUNKS, K], f8, tag="zst")
            nc.sync.dma_start(zst_t[:], zst_d[:])
            xt_t = []
            off = 0
            for b, npair in enumerate(PAIR_BLOCKS):
                cols = npair * 2 * SHARD
                t = dpool.tile([128, npair * 2, SHARD], f8, tag=f"xt{b}")
                nc.sync.dma_start(t[:], xt_d[:, off : off + cols])
                xt_t.append(t)
                off += cols
            num_t = epool.tile([K, SHARD], f8, tag="num")
            nc.scalar.dma_start(num_t[:], num_d[:])

            # -- PE: +var rank-1 matmul opens the accumulation group,
            #    then 16 fp8 DoubleRow pair matmuls, one per 2-pair
            #    block half --
            ps = pspool.tile([K, SHARD], f32, tag="ps")
            nc.tensor.matmul(
                ps[:],
                varrow_t[:, :K],
                varrow_t[:, K:],
                start=True,
                stop=False,
                skip_group_check=True,
            )
            j = 0
            for b, npair in enumerate(PAIR_BLOCKS):
                for jj in range(npair):
                    nc.tensor.matmul(
                        ps[:],
                        zst_t[:, 2 * j : 2 * j + 2, :],
                        xt_t[b][:, 2 * jj : 2 * jj + 2, :],
                        start=False,
                        stop=(j == NPAIRS - 1),
                        perf_mode=mybir.MatmulPerfMode.DoubleRow,
                        skip_group_check=True,
                    )
                    j += 1

            # -- epilogue: PSUM holds den = var + rs - num --
            rcp_t = epool.tile([K, SHARD], f32, tag="rcp")
            scr_t = epool.tile([K, SHARD], f16, tag="scr")
            red_t = epool.tile([K, 1], f32, tag="red")
            nc.vector.reciprocal_approx_fast(rcp_t[:], ps[:])
            nc.vector.scalar_tensor_tensor(
                out=scr_t[:], in0=num_t[:], scalar=1.0, in1=rcp_t[:],
                op0=mybir.AluOpType.mult, op1=mybir.AluOpType.mult,
                accum_out=red_t[:],
            )
            # cross-partition reduce on PE: out = red.T @ (-1/K * ones)
            ps1 = pspool.tile([1, 1], f32, tag="ps1")
            nc.tensor.matmul(ps1[:], red_t[:], ones_t[:], start=True, stop=True)
            nc.vector.tensor_copy(out_sb.ap(), ps1[:])

    # Post-tile out DMA (see module docstring): issued after the tile
    # block-end barrier (which guarantees out_sb is written); the body
    # does not wait for its completion semaphore.
    out_sem = nc.alloc_semaphore("out_sem")
    nc.sync.dma_start(out_d[:], out_sb.ap(), single_packet=True).then_inc(
        out_sem, 16
    )

    nc.compile()
    return nc


def _prep_inputs(zs, X, var_noise):
    """Host-side shard + layout packing (layout + dtype cast; the only
    math is diag extraction, the zs*diag elementwise product, and
    zeroing X's diagonal)."""
    import ml_dtypes

    f8 = ml_dtypes.float8_e4m3
    zs = np.ascontiguousarray(np.asarray(zs, dtype=np.float32))
    X = np.asarray(X, dtype=np.float32)
    var = np.float32(np.asarray(var_noise).reshape(()))

    diag = np.ascontiguousarray(np.diagonal(X)).astype(np.float32)
    Xz = X.copy()
    np.fill_diagonal(Xz, 0.0)

    # zst[p, m, k] = zs[k, 128m + p], replicated across cores
    zst = np.ascontiguousarray(
        zs.reshape(K, NCHUNKS, 128).transpose(2, 1, 0)
    ).astype(f8).reshape(128, NCHUNKS * K)

    varrow = np.empty((1, K + SHARD), dtype=np.float16)
    varrow[0, :K] = var
    varrow[0, K:] = 1.0

    in_maps = []
    for c in range(NCORES):
        sl = slice(c * SHARD, (c + 1) * SHARD)
        # xt[p, m, il] = Xz[c*512 + il, 128m + p]
        xt = np.ascontiguousarray(
            Xz[sl].reshape(SHARD, NCHUNKS, 128).transpose(2, 1, 0)
        ).astype(f8).reshape(128, XCOLS)
        num = (zs[:, sl] * diag[sl][None, :]).astype(f8)
        in_maps.append({"varrow": varrow, "zst": zst, "xt": xt, "num": num})
    return in_maps


def _run(in_maps, **run_kwargs):
    from concourse.bass_utils import run_bass_kernel_spmd

    if "nc" not in _CACHE:
        _CACHE["nc"] = _build()
    nc = _CACHE["nc"]
    return run_bass_kernel_spmd(
        nc, in_maps, core_ids=list(range(NCORES)), **run_kwargs
    )


def kernel(zs, X, var_noise):
    in_maps = _prep_inputs(zs, X, var_noise)
    res = None
    for attempt in range(3):
        try:
            res = _run(in_maps).results
            break
        except Exception:
            if attempt == 2:
                raise
            import time

            time.sleep(2)
    total = np.float32(0.0)
    for c in range(NCORES):
        total += res[c]["out"].astype(np.float32).sum(dtype=np.float32)
    return np.float32(total)
